# revision 97
# baseline (speedup 1.0000x reference)
"""Cross-attention Trainium2 kernel (Bass/Tile), SPMD over 8 NeuronCores.

Problem (hardcoded): x[4,4096,1024], context[4,512,768], Wq[1024,1024],
Wk[768,1024], Wv[768,1024], Wo[1024,1024], bo[1024]; 16 heads, dim 64.
    q = x@Wq; k = ctx@Wk; v = ctx@Wv (per-head 64)
    out = softmax(q k^T / 8) v;  y = out@Wo + bo
Sharding: core i -> (batch b = i//2, query half = i%2, 2048 rows), all 16
heads per core. No collectives; host concatenates the 8 output shards.

Device dataflow (bf16 operands on every matmul whose moving free dim is
narrow, f32 accumulation in PSUM):
    QT[d,n]   = Wq^T x^T        (lhsT=Wq chunk, rhs=xT chunk)
    KT[d,m]   = Wk^T ctx^T
    V[m,d]    = ctx Wv          (bf16; ones col 64 per 65-wide block)
    ET[m,n]   = exp(KT_h^T QT_h)   (bf16 in SBUF)
    O[n,65]   = ET^T V_aug      <- transposed PV: free dim is 65, not 512,
      so PE streams half the columns of the natural V^T ET orientation;
      col 64 = softmax denominator per query row (per-partition!).
    normalize: DVE reciprocal of the denom cols -> tensor_scalar_mul with
      a [128,1] per-partition scalar (no PE broadcast matmuls, no
      partition-shift DMA).
    OT[hd,n]  = dma_start_transpose(O)  -- XBAR tile transpose on the idle
      DMA engines (14ns/tile); lands directly in Wo's lhsT layout.
    y[n,c]    = OT^T Wo + bo    (Wo in bf16)
The softmax max-subtraction is skipped: scores ~ N(0,1), exp is safe in
fp32. The 1/8 scale is folded into Wq on the host.

Schedule: a short train of dummy matmuls on memset data warms the PE
p-state ramp while the first wq/x DMAs stream. Phase A runs QT(0)/KT/V
chunk-major across 8 borrowed psum slots, paced by the DMA stream. The
main loop pipelines attn(nt) | Wo(nt-1) | QT(nt+1). Wo groups split into
a j0..j6 partial at step g and a j7 finisher + bias-add + deferred y
store at step g+1, so nothing ever waits on the freshest transpose and
every DMA's producers are done when SP issues it (DMA sem waits hold the
issuing sequencer, and HWDGE completion sems ride an 8-lane ring, so both
the count and the placement of DMA instructions matter: x streams one 3D
DMA per n-tile, O transposes are 4 nk-ascending instrs covering pairs
0..6 at step 7 + pair 7 at the next step 0). The tail fills the last
pair's exp/PV/normalize latency with two rounds of Wo partials (j0..j4,
then j5..j6 once the late transposes land), then 8 one-matmul finishers
+ a PE bias outer product complete y with minimal drain.

TimelineSim: 225055 ns/core (baseline 278114).
"""

import numpy as np

import concourse.bass as bass
import concourse.mybir as mybir
import concourse.tile as tile
from concourse import bacc, library_config
from concourse.bass_utils import run_bass_kernel_spmd

F32 = mybir.dt.float32
F32R = mybir.dt.float32r
BF16 = mybir.dt.bfloat16

# PSUM pool split (8 banks total)
PSA = 2   # projection psum slots (QT/Wo groups share)
PSC = 2   # 2-bank score/exp slots
PPV = 2   # PV accumulation slots

B, N, C = 4, 4096, 1024
M, CC = 512, 768
H, D = 16, 64
INNER = H * D          # 1024
NPC = N // 2           # 2048 query rows per core
NT = NPC // 512        # 4 n-tiles of 512
NCHUNK_Q = C // 128    # 8 contraction chunks for Q proj
NCHUNK_K = CC // 128   # 6 contraction chunks for K/V proj
NPAIR = H // 2         # 8 head pairs (2 heads stacked per 128 partitions)
NMC = M // 128         # 4 key chunks
VBLK = D + 1           # 65: [V(64) | ones] per (mc, head) block
NDUMMY = 6             # PE warmup matmuls during the startup DMA window


def build_nc() -> bass.Bass:
    nc = bacc.Bacc("TRN2", target_bir_lowering=False, debug=False, num_devices=8)

    # Startup-critical streams are bf16: halves the serial DMA window before
    # the attention pipeline can roll. PSUM accumulation stays f32.
    xT = nc.dram_tensor("xT", [C, NPC], BF16, kind="ExternalInput")
    ctxT = nc.dram_tensor("ctxT", [CC, M], BF16, kind="ExternalInput")
    wq = nc.dram_tensor("wq", [C, INNER], BF16, kind="ExternalInput")
    wk = nc.dram_tensor("wk", [CC, INNER], BF16, kind="ExternalInput")
    wv = nc.dram_tensor("wv", [CC, INNER], BF16, kind="ExternalInput")
    wo = nc.dram_tensor("wo", [INNER, C], BF16, kind="ExternalInput")
    bo = nc.dram_tensor("bo", [1, C], F32, kind="ExternalInput")
    y = nc.dram_tensor("y", [NPC, C], BF16, kind="ExternalOutput")

    with tile.TileContext(nc) as tc:
        with (
            tc.tile_pool(name="persist", bufs=1) as pp,
            tc.tile_pool(name="psA", bufs=PSA, space="PSUM") as ps_a,
            tc.tile_pool(name="psSC", bufs=PSC, space="PSUM") as ps_sc,
            tc.tile_pool(name="psPV", bufs=PPV, space="PSUM") as ps_pv,
        ):
            # ---- persistent SBUF ----
            wq_sb = pp.tile([128, NCHUNK_Q * INNER], BF16)   # 16KB/part
            wo_sb = pp.tile([128, NCHUNK_Q * C], BF16)       # 16KB/part
            kt_sb = pp.tile([128, NPAIR * M], F32R)          # 16KB/part
            v_sb = pp.tile([128, NMC * H * VBLK], BF16)      # 8.1KB/part
            ones_f32 = pp.tile([128, 128], F32)
            ones_r = pp.tile([1, 128], F32R)
            bo_sb = pp.tile([128, C], F32)
            bo_row = pp.tile([1, C], F32)
            bo_row_r = pp.tile([1, C], F32R)

            nc.gpsimd.memset(ones_f32[:], 1.0)

            # PE p-state warmup: burn the cold/mid ramp on dummy matmuls
            # (f32 rhs = 4 cycles/row so few instructions cover the ~4us
            # startup DMA window); the real projections then issue into a
            # warm, already-ramped array with no leading idle gap.
            dummy_ps = ps_a.tile([128, 512], F32, tag="psA", name="warm")
            for _ in range(NDUMMY):
                nc.tensor.matmul(
                    dummy_ps[:, 0:128], ones_f32[:], ones_f32[:],
                    start=True, stop=True,
                )

            # memset can't write f32r/bf16 V directly; bounce through ACT
            nc.scalar.copy(  # ones column (col 64) of every (mc, head) block
                out=v_sb[:].rearrange("p (b q) -> p b q", q=VBLK)[:, :, D : D + 1],
                in_=ones_f32[:, 0 : NMC * H].rearrange("p (b q) -> p b q", q=1),
            )
            nc.scalar.copy(out=ones_r[:], in_=ones_f32[0:1, :])

            with (
                tc.tile_pool(name="xt", bufs=10) as xp,
                tc.tile_pool(name="qt", bufs=10) as qp,
            ):
                xt_t = {}
                qt_t = {}
                otb_t = {}   # per-nt transposed-O buffers [128, 8*512] bf16
                osb_t = {}   # per-nt pre-transpose O [128, 4096] bf16, nk-major
                osb7_t = {}  # pair-7 contiguous [128, 512] variant (1-instr transpose)

                def emit_x_chunk(nt, c):
                    if c == 0:
                        xt_t[nt] = []
                    t = xp.tile([128, 512], BF16, tag="xt", name=f"xt{nt}_{c}")
                    nc.sync.dma_start(
                        out=t[:],
                        in_=xT[c * 128 : (c + 1) * 128, nt * 512 : (nt + 1) * 512],
                    )
                    xt_t[nt].append(t[:])

                def emit_x_nt(nt):
                    # one 3D-AP DMA for a whole n-tile's x: 8x fewer HWDGE
                    # trips, far less pressure on the 8-lane DMAHW sem ring
                    big = xp.tile([128, NCHUNK_Q * 512], BF16, tag="xtb",
                                  name=f"xtb{nt}", bufs=2)
                    nc.sync.dma_start(
                        out=big[:],
                        in_=xT[:, nt * 512 : (nt + 1) * 512].rearrange(
                            "(c p) n -> p c n", p=128
                        ),
                    )
                    xt_t[nt] = [big[:, c * 512 : (c + 1) * 512]
                                for c in range(NCHUNK_Q)]

                def emit_qt_pair(nt, j):
                    if j == 0:
                        qt_t[nt] = {}
                    qt_t[nt][j] = qp.tile(
                        [128, 512], F32R, tag="qt", name=f"qt{nt}_{j}"
                    )
                    qt = qt_t[nt][j]
                    xt = xt_t[nt]
                    qps = ps_a.tile([128, 512], F32, tag="psA")
                    for c in range(NCHUNK_Q):
                        nc.tensor.matmul(
                            qps[:],
                            wq_sb[:, c * INNER + j * 128 : c * INNER + (j + 1) * 128],
                            xt[c],
                            start=(c == 0),
                            stop=(c == NCHUNK_Q - 1),
                        )
                    # DVE (not ACT) copy: keeps ACT free for the exp train
                    nc.vector.tensor_copy(qt[:], qps[:])

                # ---- phase A: streams + projections, PE starts ~2us in ----
                with tc.tile_pool(name="setup", bufs=1) as sp:
                    wk_sb = sp.tile([128, NCHUNK_K * INNER], BF16)
                    wv_sb = sp.tile([128, NCHUNK_K * INNER], BF16)
                    ctx_sb = sp.tile([128, NCHUNK_K * M], BF16)

                    # x0[0] first and wq[0] in halves so the first QT
                    # matmuls fire as early as possible
                    emit_x_chunk(0, 0)
                    # j0 slice first: the very first QT matmul needs only
                    # these 128 columns, so it fires one DMA-slot earlier
                    nc.sync.dma_start(
                        out=wq_sb[:, 0:128], in_=wq[0:128, 0:128],
                    )
                    nc.sync.dma_start(
                        out=wq_sb[:, 128:512], in_=wq[0:128, 128:512],
                    )
                    nc.sync.dma_start(
                        out=wq_sb[:, 512:1024], in_=wq[0:128, 512:1024],
                    )
                    for c in range(1, NCHUNK_Q):  # wq + x0 interleaved
                        nc.sync.dma_start(
                            out=wq_sb[:, c * INNER : (c + 1) * INNER],
                            in_=wq[c * 128 : (c + 1) * 128, :],
                        )
                        emit_x_chunk(0, c)
                    for c in range(NCHUNK_K):  # wk + ctx + wv interleaved
                        nc.sync.dma_start(
                            out=wk_sb[:, c * INNER : (c + 1) * INNER],
                            in_=wk[c * 128 : (c + 1) * 128, :],
                        )
                        nc.sync.dma_start(
                            out=ctx_sb[:, c * M : (c + 1) * M],
                            in_=ctxT[c * 128 : (c + 1) * 128, :],
                        )
                        nc.sync.dma_start(
                            out=wv_sb[:, c * INNER : (c + 1) * INNER],
                            in_=wv[c * 128 : (c + 1) * 128, :],
                        )
                    nc.sync.dma_start(out=bo_row[:], in_=bo[:, :])
                    for c in range(NCHUNK_Q):
                        emit_x_chunk(1, c)

                    # 8 concurrent [128,512] psum accumulators drawn from the
                    # three phase-B pools; the pool rings give safe reuse
                    # ordering. Chunk-major (c-outer) emission lets the PE
                    # pace every projection with the DMA stream instead of
                    # idling while one pair's chunks trickle in.
                    def alloc_slots8():
                        sc0 = ps_sc.tile([128, 1024], F32, tag="sc")
                        sc1 = ps_sc.tile([128, 1024], F32, tag="sc")
                        a0 = ps_a.tile([128, 512], F32, tag="psA")
                        a1 = ps_a.tile([128, 512], F32, tag="psA")
                        v0 = ps_pv.tile([128, 512], F32, tag="pv")
                        v1 = ps_pv.tile([128, 512], F32, tag="pv")
                        return [
                            sc0[:, 0:512], sc0[:, 512:1024],
                            sc1[:, 0:512], sc1[:, 512:1024],
                            a0[:], a1[:], v0[:], v1[:],
                        ]

                    # QT(0): paced by the wq/x0 stream
                    slots = alloc_slots8()
                    qt_t[0] = {}
                    for c in range(NCHUNK_Q):
                        for j in range(NPAIR):
                            nc.tensor.matmul(
                                slots[j],
                                wq_sb[:, c * INNER + j * 128 : c * INNER + (j + 1) * 128],
                                xt_t[0][c],
                                start=(c == 0),
                                stop=(c == NCHUNK_Q - 1),
                            )
                    for j in range(NPAIR):
                        qt_t[0][j] = qp.tile([128, 512], F32R, tag="qt", name=f"qt0_{j}")
                        if j % 2 == 0:
                            nc.vector.tensor_copy(qt_t[0][j][:], slots[j])
                        else:
                            nc.scalar.copy(out=qt_t[0][j][:], in_=slots[j])

                    # KT per head pair [128 (2 heads d), 512 m]: wk/ctx-paced
                    slots = alloc_slots8()
                    for c in range(NCHUNK_K):
                        for j in range(NPAIR):
                            nc.tensor.matmul(
                                slots[j],
                                wk_sb[:, c * INNER + j * 128 : c * INNER + (j + 1) * 128],
                                ctx_sb[:, c * M : (c + 1) * M],
                                start=(c == 0),
                                stop=(c == NCHUNK_K - 1),
                            )
                    for j in range(NPAIR):
                        # alternate engines: the serial copy chain is what
                        # frees the V projection's psum slots
                        if j % 2 == 0:
                            nc.scalar.copy(
                                out=kt_sb[:, j * M : (j + 1) * M], in_=slots[j]
                            )
                        else:
                            nc.vector.tensor_copy(
                                kt_sb[:, j * M : (j + 1) * M], slots[j]
                            )

                    # V natural [m, d] with ones col, 8 (mc, hf) units. wv is
                    # resident by now, so per-unit order paces with the KT
                    # copies that progressively free the psum ring slots.
                    slots = alloc_slots8()
                    units = [(mc, hf) for hf in range(2) for mc in range(NMC)]
                    for u, (mc, hf) in enumerate(units):
                        for c in range(NCHUNK_K):
                            nc.tensor.matmul(
                                slots[u],
                                ctx_sb[:, c * M + mc * 128 : c * M + (mc + 1) * 128],
                                wv_sb[:, c * INNER + hf * 512 : c * INNER + (hf + 1) * 512],
                                start=(c == 0),
                                stop=(c == NCHUNK_K - 1),
                            )
                        base = mc * H * VBLK + hf * 8 * VBLK
                        nc.vector.tensor_copy(
                            v_sb[:, base : base + 8 * VBLK].rearrange(
                                "p (h q) -> p h q", q=VBLK
                            )[:, :, 0:D],
                            slots[u].rearrange("p (h q) -> p h q", q=D),
                        )

                    # bias broadcast on the idle Pool engine
                    nc.gpsimd.partition_broadcast(bo_sb[:, :], bo_row[0:1, :])
                    nc.scalar.copy(out=bo_row_r[:], in_=bo_row[:])

                # ---- phase B: software-pipelined across 512-query tiles ----
                with (
                    tc.tile_pool(name="et", bufs=6) as ep,
                    tc.tile_pool(name="osb", bufs=2) as op,
                    tc.tile_pool(name="otb", bufs=2) as obp,
                    tc.tile_pool(name="rows", bufs=4) as rp,
                    tc.tile_pool(name="ysb", bufs=4) as yp,
                ):
                    for h in range(2):
                        nc.sync.dma_start(
                            out=wo_sb[:, h * 4 * C : (h + 1) * 4 * C],
                            in_=wo[h * 512 : (h + 1) * 512, :].rearrange(
                                "(c p) f -> p c f", p=128
                            ),
                        )

                    # Wo groups are split: the j0..j6 partial runs at step g,
                    # the j7 contribution + bias + y store at step g+1. No Wo
                    # matmul ever waits on the freshly-transposed pair 7, and
                    # the y DMA's producers are long done when SP issues it
                    # (DMA waits hold the sequencer in this machine).
                    wo_pend = {}

                    def emit_wo_partial(nt, g):
                        ns, cg = g // 2, g % 2
                        otb = otb_t[nt]
                        yps = ps_a.tile([128, 512], F32, tag="psA")
                        for j in range(NPAIR - 1):
                            nc.tensor.matmul(
                                yps[:],
                                otb[:, (ns * 8 + j) * 128 : (ns * 8 + j + 1) * 128],
                                wo_sb[:, j * C + cg * 512 : j * C + (cg + 1) * 512],
                                start=(j == 0),
                                stop=False,
                            )
                        wo_pend[(nt, g)] = yps

                    y_fifo = []
                    wo_ysb = {}

                    def emit_wo_finish(nt, g):
                        ns, cg = g // 2, g % 2
                        otb = otb_t[nt]
                        yps = wo_pend.pop((nt, g))
                        j = NPAIR - 1
                        nc.tensor.matmul(
                            yps[:],
                            otb[:, (ns * 8 + j) * 128 : (ns * 8 + j + 1) * 128],
                            wo_sb[:, j * C + cg * 512 : j * C + (cg + 1) * 512],
                            start=False,
                            stop=True,
                        )
                        # Both cg halves of a row block share one [128,1024]
                        # ysb and go out as a single store, deferred a step
                        # past the second add: half the HWDGE trips, and the
                        # DMA's sem wait (which holds the SP sequencer) is
                        # already satisfied when it issues.
                        if cg == 0:
                            wo_ysb[(nt, ns)] = yp.tile(
                                [128, 1024], BF16, tag="ysb", name=f"ysb{nt}_{ns}"
                            )
                        ysb = wo_ysb[(nt, ns)]
                        nc.vector.tensor_add(
                            ysb[:, cg * 512 : (cg + 1) * 512],
                            yps[:],
                            bo_sb[:, cg * 512 : (cg + 1) * 512],
                        )
                        if cg == 1:
                            y_fifo.append((nt, ns, wo_ysb.pop((nt, ns))))

                    def y_flush(k):
                        for _ in range(min(k, len(y_fifo))):
                            nt, ns, ysb = y_fifo.pop(0)
                            nc.sync.dma_start(
                                out=y[
                                    nt * 512 + ns * 128 : nt * 512 + (ns + 1) * 128,
                                    :,
                                ],
                                in_=ysb[:],
                            )

                    def emit_attn_scores(nt, j, last=False):
                        qt = qt_t[nt][j]
                        # scoresT both halves, mc-interleaved: halves hit PE
                        # row strips 0-63 / 64-127. For the last pair, emit
                        # odd-half-major and split each exp into two half-tile
                        # activations so the tail PV paces per m-chunk instead
                        # of waiting the full 4us exp train.
                        ets = [[], []]
                        order = (
                            [(mcp, half) for half in (1, 0) for mcp in range(2)]
                            if last
                            else [(mcp, half) for mcp in range(2) for half in range(2)]
                        )
                        for mcp, half in order:
                            p0, p1 = half * 64, half * 64 + 64
                            # two m-chunks share a 2-bank psum tile so one
                            # ACT exp covers both (fixed-cost amortize)
                            scps = ps_sc.tile([128, 1024], F32, tag="sc")
                            for k in range(2):
                                mc = 2 * mcp + k
                                nc.tensor.matmul(
                                    scps[:, k * 512 : (k + 1) * 512],
                                    kt_sb[p0:p1, j * M + mc * 128 : j * M + (mc + 1) * 128],
                                    qt[p0:p1, :],
                                    start=True,
                                    stop=True,
                                )
                            et = ep.tile([128, 1024], BF16, tag="et")
                            if last:
                                for k in range(2):
                                    nc.scalar.activation(
                                        et[:, k * 512 : (k + 1) * 512],
                                        scps[:, k * 512 : (k + 1) * 512],
                                        mybir.ActivationFunctionType.Exp,
                                    )
                            else:
                                nc.scalar.activation(
                                    et[:], scps[:], mybir.ActivationFunctionType.Exp
                                )
                            ets[half].append(et)
                        return ets

                    def emit_attn_pvT(nt, j, ets):
                        # Transposed PV per head: O[n128, 65] = sum_mc
                        # ET_slice^T V_aug. Free dim 65 -> the PE streams
                        # ~half the columns of the natural orientation, and
                        # col 64 lands the softmax denominator per PARTITION
                        # so normalization is a DVE tensor_scalar. O is one
                        # nk-major [128, 4096] tile per nt so a single XBAR
                        # transpose instruction covers the whole n-tile.
                        if j == 0:
                            osb_t[nt] = op.tile(
                                [128, 4096], BF16, tag="osb", name=f"osb{nt}"
                            )
                        single = j == JL or (nt == NT - 1 and j >= JL - 2)
                        if single:
                            # transpose-latency-critical pairs (every pair 7,
                            # plus the last n-tile's pairs 5/6) land in their
                            # own contiguous nk-major buffer so each whole
                            # transpose is ONE instruction instead of four
                            osb7_t[(nt, j)] = op.tile(
                                [128, 512], BF16, tag="osb7",
                                name=f"osbp{nt}_{j}", bufs=3,
                            )
                        osb = osb_t[nt]
                        for half in (0, 1):
                            h = 2 * j + half
                            pv = ps_pv.tile([128, 512], F32, tag="pv")
                            for nk in range(4):
                                for mc in range(NMC):
                                    vb = mc * H * VBLK + h * VBLK
                                    etap = ets[half][mc // 2][
                                        :, (mc % 2) * 512 + nk * 128
                                        : (mc % 2) * 512 + (nk + 1) * 128
                                    ]
                                    nc.tensor.matmul(
                                        pv[:, nk * VBLK : (nk + 1) * VBLK],
                                        etap,
                                        v_sb[:, vb : vb + VBLK],
                                        start=(mc == 0),
                                        stop=(mc == NMC - 1),
                                    )
                            # normalize: recip of the 4 denom cols, then 4
                            # per-partition scalar muls into O_sb (bf16)
                            pvv = pv[:, 0 : 4 * VBLK].rearrange(
                                "p (a q) -> p a q", q=VBLK
                            )
                            rr = rp.tile([128, 4], F32, tag="rows")
                            with nc.allow_low_precision(reason="denoms>0"):
                                nc.vector.reciprocal(
                                    rr[:].rearrange("p (a q) -> p a q", q=1),
                                    pvv[:, :, D : D + 1],
                                )
                            if single:
                                nout = osb7_t[(nt, j)][:].rearrange(
                                    "p (a q) -> p a q", q=128
                                )[:, :, half * 64 : (half + 1) * 64]
                            else:
                                nout = osb[:].rearrange(
                                    "p (a q) -> p a q", q=1024
                                )[:, :, h * 64 : (h + 1) * 64]
                            nc.vector.tensor_mul(
                                nout,
                                pvv[:, :, 0:D],
                                rr[:].rearrange("p (a q) -> p a q", q=1)
                                .broadcast_to((128, 4, D)),
                            )

                    def emit_transpose(nt, j0, j1):
                        # O[n, hd] -> OT[hd, n] on the DMA engines' XBAR. The
                        # otb layout is (nk, j)-major ([hd, (nk*8+j)*128+n])
                        # so a FULL n-tile is ONE instruction (3D out AP:
                        # out[hd, g, n] = in[n, g*128+hd]); pair ranges are
                        # one instr per n-chunk. Wo reads its lhsT slice at
                        # (ns*8+j)*128, which is just a re-indexing.
                        if nt not in otb_t:
                            otb_t[nt] = obp.tile(
                                [128, NPAIR * 512], BF16, tag="otb",
                                name=f"otb{nt}",
                            )
                        otb = otb_t[nt]
                        if j1 - j0 == 1 and (nt, j0) in osb7_t:
                            # one instruction from the pair's contiguous
                            # nk-major buffer; out blocks stride 1024 apart
                            nc.sync.dma_start_transpose(
                                out=otb[:].rearrange("p (a q) -> p a q", q=1024)[
                                    :, :, j0 * 128 : (j0 + 1) * 128
                                ],
                                in_=osb7_t[(nt, j0)][:],
                            )
                            return
                        osb = osb_t[nt]
                        for nk in range(4):
                            nc.sync.dma_start_transpose(
                                out=otb[
                                    :, (nk * 8 + j0) * 128 : (nk * 8 + j1) * 128
                                ].rearrange("p (g n) -> p g n", n=128),
                                in_=osb[:, nk * 1024 + j0 * 128 : nk * 1024 + j1 * 128],
                            )

                    JL = NPAIR - 1
                    for nt in range(NT):
                        if nt + 2 < NT:
                            emit_x_nt(nt + 2)
                        for j in range(NPAIR):
                            if nt == NT - 1 and j == JL:
                                break  # tail handled below
                            ets = emit_attn_scores(nt, j)
                            # the previous group's j7 finisher goes right
                            # after the scores: its DVE add is what recycles
                            # the psA slot the upcoming partial needs, so it
                            # must not queue behind this pair's normalize
                            if nt > 0:
                                if j >= 1:
                                    emit_wo_finish(nt - 1, j - 1)
                                elif nt >= 2:
                                    emit_wo_finish(nt - 2, JL)
                                y_flush(1)
                            emit_attn_pvT(nt, j, ets)
                            # Pairs 0..6 transpose at their own nt's step 7
                            # (all normalized by then), pair 7 at the next
                            # nt's step 0 — so the boundary Wo partial (which
                            # reads j0..6) never waits on an in-flight
                            # transpose, and the j7 finisher has a full step
                            # of slack. nk-ascending instrs mean group g only
                            # needs instr nk=g//2.
                            if j == 0 and nt > 0:
                                emit_transpose(nt - 1, JL, NPAIR)  # pair 7
                            if j == JL and nt < NT - 1:
                                emit_transpose(nt, 0, JL)  # pairs 0..6
                            if nt == NT - 1 and j == JL - 2:
                                emit_transpose(nt, 0, JL - 2)  # pairs 0..4
                            if nt == NT - 1 and j == JL - 1:
                                emit_transpose(nt, JL - 2, JL - 1)  # pair 5
                            if nt + 1 < NT:
                                emit_qt_pair(nt + 1, j)
                            if nt > 0:
                                emit_wo_partial(nt - 1, j)
                    emit_transpose(NT - 1, JL - 1, JL)  # pair 6 of the last nt
                    # dangling Wo(2) work the broken step (3,7) would have run
                    emit_wo_finish(NT - 2, JL - 1)
                    y_flush(2)
                    emit_wo_partial(NT - 2, JL)
                    emit_wo_finish(NT - 2, JL)
                    y_flush(1)

                    # ---- tail: last pair of the last n-tile ----
                    # Fill the exp/normalize latency with j0..j6 Wo partials.
                    # Only 2 psA slots exist, so the remaining partials borrow
                    # the score/PV psum rings (idle from here on); all eight
                    # j7 finishers then fire back-to-back once ot(3,7) lands.
                    ets = emit_attn_scores(NT - 1, JL)
                    wps = []
                    for g in range(2):
                        wt = ps_a.tile([128, 512], F32, tag="psA", name=f"tailA{g}")
                        wps.append(wt[:])

                    def tail_partial(g, j0=0, j1=NPAIR - 1):
                        ns, cg = g // 2, g % 2
                        otb = otb_t[NT - 1]
                        for j in range(j0, j1):
                            nc.tensor.matmul(
                                wps[g],
                                otb[:, (ns * 8 + j) * 128 : (ns * 8 + j + 1) * 128],
                                wo_sb[:, j * C + cg * 512 : j * C + (cg + 1) * 512],
                                start=(j == 0),
                                stop=False,
                            )


                    tail_ysb = {}

                    def tail_finish(g):
                        # j7 contribution, then bias via a PE outer product
                        # into the same accumulation. cg pairs share one
                        # [128,1024] ysb so each row-block needs a single
                        # HWDGE hold at the very end instead of two.
                        ns, cg = g // 2, g % 2
                        otb = otb_t[NT - 1]
                        nc.tensor.matmul(
                            wps[g],
                            otb[:, (ns * 8 + JL) * 128 : (ns * 8 + JL + 1) * 128],
                            wo_sb[:, JL * C + cg * 512 : JL * C + (cg + 1) * 512],
                            start=False,
                            stop=False,
                        )
                        nc.tensor.matmul(
                            wps[g],
                            ones_r[0:1, :],
                            bo_row_r[0:1, cg * 512 : (cg + 1) * 512],
                            start=False,
                            stop=True,
                        )
                        if ns not in tail_ysb:
                            # share the main ysb ring (4 deep) so this
                            # allocation never waits on an earlier y store
                            tail_ysb[ns] = yp.tile(
                                [128, 1024], BF16, tag="ysb", name=f"ysbw{ns}"
                            )
                        ysb = tail_ysb[ns]
                        # spread the PSUM->SBUF hops so neither engine
                        # serializes the closing stores (ACT takes the g5/g7
                        # halves that gate the last two row blocks)
                        if g in (0, 2, 4, 5, 7):
                            nc.scalar.copy(
                                out=ysb[:, cg * 512 : (cg + 1) * 512], in_=wps[g]
                            )
                        else:
                            nc.vector.tensor_copy(
                                ysb[:, cg * 512 : (cg + 1) * 512], wps[g]
                            )
                        if cg == 1:
                            nc.sync.dma_start(
                                out=y[
                                    (NT - 1) * 512 + ns * 128 : (NT - 1) * 512 + (ns + 1) * 128,
                                    :,
                                ],
                                in_=ysb[:],
                            )

                    # Partials run in two rounds: j0..j4 (whose OT chunks are
                    # long since resident) fill the PE while the last pair's
                    # exp/PV/normalize chain runs; j5..j6 follow once the
                    # late transposes land. Keeping the rounds as separate
                    # accumulation bursts means no in-order PE chain ever
                    # head-of-line blocks on a transpose still in flight.
                    tail_partial(0, 0, NPAIR - 3)
                    tail_partial(1, 0, NPAIR - 3)
                    emit_attn_pvT(NT - 1, JL, ets)
                    emit_transpose(NT - 1, JL, NPAIR)
                    # sc/pv ring slots must be claimed after the last pair's
                    # scores and PV tiles so ring reuse ordering cannot
                    # deadlock against the finishers
                    sc0 = ps_sc.tile([128, 1024], F32, tag="sc")
                    sc1 = ps_sc.tile([128, 1024], F32, tag="sc")
                    wps += [sc0[:, 0:512], sc0[:, 512:1024],
                            sc1[:, 0:512], sc1[:, 512:1024]]
                    for g in range(2):
                        wt = ps_pv.tile([128, 512], F32, tag="pv", name=f"tailPV{g}")
                        wps.append(wt[:])
                    for g in range(2, NPAIR):
                        tail_partial(g, 0, NPAIR - 3)
                    # round 2 + finishers, interleaved so each finisher fires
                    # as soon as its ot(3,7) chunk lands
                    tail_partial(0, NPAIR - 3, NPAIR - 1)
                    tail_partial(1, NPAIR - 3, NPAIR - 1)
                    for g in range(2, NPAIR):
                        tail_partial(g, NPAIR - 3, NPAIR - 1)
                        tail_finish(g - 2)
                    tail_finish(NPAIR - 2)
                    tail_finish(NPAIR - 1)

    nc.compile()
    return nc


_NC_CACHE = None


def kernel(x, context, Wq, Wk, Wv, Wo, bo, _trace=False, _trace_kwargs=None):
    global _NC_CACHE
    if _NC_CACHE is None:
        _NC_CACHE = build_nc()
    nc = _NC_CACHE

    bf16 = mybir.dt.np(BF16)
    x = np.asarray(x, np.float32)
    context = np.asarray(context, np.float32)
    wq_s = (np.asarray(Wq, np.float32) * np.float32(D**-0.5)).astype(bf16)
    wk = np.asarray(Wk, np.float32).astype(bf16)
    wv = np.asarray(Wv, np.float32).astype(bf16)
    wo = np.asarray(Wo, np.float32).astype(bf16)
    bo2 = np.asarray(bo, np.float32).reshape(1, C)

    in_maps = []
    for i in range(8):
        b, hf = i // 2, i % 2
        in_maps.append(
            {
                "xT": np.ascontiguousarray(x[b, hf * NPC : (hf + 1) * NPC, :].T).astype(bf16),
                "ctxT": np.ascontiguousarray(context[b].T).astype(bf16),
                "wq": wq_s,
                "wk": wk,
                "wv": wv,
                "wo": wo,
                "bo": bo2,
            }
        )

    kw = {}
    if _trace:
        kw = dict(trace=True, trace_kwargs=_trace_kwargs or {})
    res = run_bass_kernel_spmd(nc, in_maps, list(range(8)), **kw)

    out = np.empty((B, N, C), np.float32)
    for i in range(8):
        b, hf = i // 2, i % 2
        out[b, hf * NPC : (hf + 1) * NPC, :] = np.asarray(
            res.results[i]["y"], dtype=np.float32
        )
    if _trace:
        return out, res
    return out


# revision 98
# speedup vs baseline: 1.0022x; 1.0022x over previous
"""Cross-attention Trainium2 kernel (Bass/Tile), SPMD over 8 NeuronCores.

Problem (hardcoded): x[4,4096,1024], context[4,512,768], Wq[1024,1024],
Wk[768,1024], Wv[768,1024], Wo[1024,1024], bo[1024]; 16 heads, dim 64.
    q = x@Wq; k = ctx@Wk; v = ctx@Wv (per-head 64)
    out = softmax(q k^T / 8) v;  y = out@Wo + bo
Sharding: core i -> (batch b = i//2, query half = i%2, 2048 rows), all 16
heads per core. No collectives; host concatenates the 8 output shards.

Device dataflow (bf16 operands on every matmul whose moving free dim is
narrow, f32 accumulation in PSUM):
    QT[d,n]   = Wq^T x^T        (lhsT=Wq chunk, rhs=xT chunk)
    KT[d,m]   = Wk^T ctx^T
    V[m,d]    = ctx Wv          (bf16; ones col 64 per 65-wide block)
    ET[m,n]   = exp(KT_h^T QT_h)   (bf16 in SBUF)
    O[n,65]   = ET^T V_aug      <- transposed PV: free dim is 65, not 512,
      so PE streams half the columns of the natural V^T ET orientation;
      col 64 = softmax denominator per query row (per-partition!).
    normalize: DVE reciprocal of the denom cols -> tensor_scalar_mul with
      a [128,1] per-partition scalar (no PE broadcast matmuls, no
      partition-shift DMA).
    OT[hd,n]  = dma_start_transpose(O)  -- XBAR tile transpose on the idle
      DMA engines (14ns/tile); lands directly in Wo's lhsT layout.
    y[n,c]    = OT^T Wo + bo    (Wo in bf16)
The softmax max-subtraction is skipped: scores ~ N(0,1), exp is safe in
fp32. The 1/8 scale is folded into Wq on the host.

Schedule: a short train of dummy matmuls on memset data warms the PE
p-state ramp while the first wq/x DMAs stream. Phase A runs QT(0)/KT/V
chunk-major across 8 borrowed psum slots, paced by the DMA stream. The
main loop pipelines attn(nt) | Wo(nt-1) | QT(nt+1). Wo groups split into
a j0..j6 partial at step g and a j7 finisher + bias-add + deferred y
store at step g+1, so nothing ever waits on the freshest transpose and
every DMA's producers are done when SP issues it (DMA sem waits hold the
issuing sequencer, and HWDGE completion sems ride an 8-lane ring, so both
the count and the placement of DMA instructions matter: x streams one 3D
DMA per n-tile, O transposes are 4 nk-ascending instrs covering pairs
0..6 at step 7 + pair 7 at the next step 0). The tail fills the last
pair's exp/PV/normalize latency with two rounds of Wo partials (j0..j4,
then j5..j6 once the late transposes land), then 8 one-matmul finishers
+ a PE bias outer product complete y with minimal drain.

TimelineSim: 225055 ns/core (baseline 278114).
"""

import numpy as np

import concourse.bass as bass
import concourse.mybir as mybir
import concourse.tile as tile
from concourse import bacc, library_config
from concourse.bass_utils import run_bass_kernel_spmd

F32 = mybir.dt.float32
F32R = mybir.dt.float32r
BF16 = mybir.dt.bfloat16

# PSUM pool split (8 banks total)
PSA = 2   # projection psum slots (QT/Wo groups share)
PSC = 2   # 2-bank score/exp slots
PPV = 2   # PV accumulation slots

B, N, C = 4, 4096, 1024
M, CC = 512, 768
H, D = 16, 64
INNER = H * D          # 1024
NPC = N // 2           # 2048 query rows per core
NT = NPC // 512        # 4 n-tiles of 512
NCHUNK_Q = C // 128    # 8 contraction chunks for Q proj
NCHUNK_K = CC // 128   # 6 contraction chunks for K/V proj
NPAIR = H // 2         # 8 head pairs (2 heads stacked per 128 partitions)
NMC = M // 128         # 4 key chunks
VBLK = D + 1           # 65: [V(64) | ones] per (mc, head) block
NDUMMY = 6             # PE warmup matmuls during the startup DMA window


def build_nc() -> bass.Bass:
    nc = bacc.Bacc("TRN2", target_bir_lowering=False, debug=False, num_devices=8)

    # Startup-critical streams are bf16: halves the serial DMA window before
    # the attention pipeline can roll. PSUM accumulation stays f32.
    xT = nc.dram_tensor("xT", [C, NPC], BF16, kind="ExternalInput")
    ctxT = nc.dram_tensor("ctxT", [CC, M], BF16, kind="ExternalInput")
    wq = nc.dram_tensor("wq", [C, INNER], BF16, kind="ExternalInput")
    wk = nc.dram_tensor("wk", [CC, INNER], BF16, kind="ExternalInput")
    wv = nc.dram_tensor("wv", [CC, INNER], BF16, kind="ExternalInput")
    wo = nc.dram_tensor("wo", [INNER, C], BF16, kind="ExternalInput")
    bo = nc.dram_tensor("bo", [1, C], F32, kind="ExternalInput")
    y = nc.dram_tensor("y", [NPC, C], BF16, kind="ExternalOutput")

    with tile.TileContext(nc) as tc:
        with (
            tc.tile_pool(name="persist", bufs=1) as pp,
            tc.tile_pool(name="psA", bufs=PSA, space="PSUM") as ps_a,
            tc.tile_pool(name="psSC", bufs=PSC, space="PSUM") as ps_sc,
            tc.tile_pool(name="psPV", bufs=PPV, space="PSUM") as ps_pv,
        ):
            # ---- persistent SBUF ----
            wq_sb = pp.tile([128, NCHUNK_Q * INNER], BF16)   # 16KB/part
            wo_sb = pp.tile([128, NCHUNK_Q * C], BF16)       # 16KB/part
            kt_sb = pp.tile([128, NPAIR * M], F32R)          # 16KB/part
            v_sb = pp.tile([128, NMC * H * VBLK], BF16)      # 8.1KB/part
            ones_f32 = pp.tile([128, 128], F32)
            ones_r = pp.tile([1, 128], F32R)
            bo_sb = pp.tile([128, C], F32)
            bo_row = pp.tile([1, C], F32)
            bo_row_r = pp.tile([1, C], F32R)

            nc.gpsimd.memset(ones_f32[:], 1.0)

            # PE p-state warmup: burn the cold/mid ramp on dummy matmuls
            # (f32 rhs = 4 cycles/row so few instructions cover the ~4us
            # startup DMA window); the real projections then issue into a
            # warm, already-ramped array with no leading idle gap.
            dummy_ps = ps_a.tile([128, 512], F32, tag="psA", name="warm")
            for _ in range(NDUMMY):
                nc.tensor.matmul(
                    dummy_ps[:, 0:128], ones_f32[:], ones_f32[:],
                    start=True, stop=True,
                )

            # memset can't write f32r/bf16 V directly; bounce through ACT
            nc.scalar.copy(  # ones column (col 64) of every (mc, head) block
                out=v_sb[:].rearrange("p (b q) -> p b q", q=VBLK)[:, :, D : D + 1],
                in_=ones_f32[:, 0 : NMC * H].rearrange("p (b q) -> p b q", q=1),
            )
            nc.scalar.copy(out=ones_r[:], in_=ones_f32[0:1, :])

            with (
                tc.tile_pool(name="xt", bufs=10) as xp,
                tc.tile_pool(name="qt", bufs=10) as qp,
            ):
                xt_t = {}
                qt_t = {}
                otb_t = {}   # per-nt transposed-O buffers [128, 8*512] bf16
                osb_t = {}   # per-nt pre-transpose O [128, 4096] bf16, nk-major
                osb7_t = {}  # pair-7 contiguous [128, 512] variant (1-instr transpose)

                def emit_x_chunk(nt, c):
                    if c == 0:
                        xt_t[nt] = []
                    t = xp.tile([128, 512], BF16, tag="xt", name=f"xt{nt}_{c}")
                    nc.sync.dma_start(
                        out=t[:],
                        in_=xT[c * 128 : (c + 1) * 128, nt * 512 : (nt + 1) * 512],
                    )
                    xt_t[nt].append(t[:])

                def emit_x_nt(nt):
                    # one 3D-AP DMA for a whole n-tile's x: 8x fewer HWDGE
                    # trips, far less pressure on the 8-lane DMAHW sem ring
                    big = xp.tile([128, NCHUNK_Q * 512], BF16, tag="xtb",
                                  name=f"xtb{nt}", bufs=2)
                    nc.sync.dma_start(
                        out=big[:],
                        in_=xT[:, nt * 512 : (nt + 1) * 512].rearrange(
                            "(c p) n -> p c n", p=128
                        ),
                    )
                    xt_t[nt] = [big[:, c * 512 : (c + 1) * 512]
                                for c in range(NCHUNK_Q)]

                def emit_qt_pair(nt, j):
                    if j == 0:
                        qt_t[nt] = {}
                    qt_t[nt][j] = qp.tile(
                        [128, 512], F32R, tag="qt", name=f"qt{nt}_{j}"
                    )
                    qt = qt_t[nt][j]
                    xt = xt_t[nt]
                    qps = ps_a.tile([128, 512], F32, tag="psA")
                    for c in range(NCHUNK_Q):
                        nc.tensor.matmul(
                            qps[:],
                            wq_sb[:, c * INNER + j * 128 : c * INNER + (j + 1) * 128],
                            xt[c],
                            start=(c == 0),
                            stop=(c == NCHUNK_Q - 1),
                        )
                    # DVE (not ACT) copy: keeps ACT free for the exp train
                    nc.vector.tensor_copy(qt[:], qps[:])

                # ---- phase A: streams + projections, PE starts ~2us in ----
                with tc.tile_pool(name="setup", bufs=1) as sp:
                    wk_sb = sp.tile([128, NCHUNK_K * INNER], BF16)
                    wv_sb = sp.tile([128, NCHUNK_K * INNER], BF16)
                    ctx_sb = sp.tile([128, NCHUNK_K * M], BF16)

                    # x0[0] first and wq[0] in halves so the first QT
                    # matmuls fire as early as possible
                    emit_x_chunk(0, 0)
                    for h in range(2):
                        nc.sync.dma_start(
                            out=wq_sb[:, h * 512 : (h + 1) * 512],
                            in_=wq[0:128, h * 512 : (h + 1) * 512],
                        )
                    for c in range(1, NCHUNK_Q):  # wq + x0 interleaved
                        nc.sync.dma_start(
                            out=wq_sb[:, c * INNER : (c + 1) * INNER],
                            in_=wq[c * 128 : (c + 1) * 128, :],
                        )
                        emit_x_chunk(0, c)
                    for c in range(NCHUNK_K):  # wk + ctx + wv interleaved
                        nc.sync.dma_start(
                            out=wk_sb[:, c * INNER : (c + 1) * INNER],
                            in_=wk[c * 128 : (c + 1) * 128, :],
                        )
                        nc.sync.dma_start(
                            out=ctx_sb[:, c * M : (c + 1) * M],
                            in_=ctxT[c * 128 : (c + 1) * 128, :],
                        )
                        nc.sync.dma_start(
                            out=wv_sb[:, c * INNER : (c + 1) * INNER],
                            in_=wv[c * 128 : (c + 1) * 128, :],
                        )
                    nc.sync.dma_start(out=bo_row[:], in_=bo[:, :])
                    for c in range(NCHUNK_Q):
                        emit_x_chunk(1, c)

                    # 8 concurrent [128,512] psum accumulators drawn from the
                    # three phase-B pools; the pool rings give safe reuse
                    # ordering. Chunk-major (c-outer) emission lets the PE
                    # pace every projection with the DMA stream instead of
                    # idling while one pair's chunks trickle in.
                    def alloc_slots8():
                        sc0 = ps_sc.tile([128, 1024], F32, tag="sc")
                        sc1 = ps_sc.tile([128, 1024], F32, tag="sc")
                        a0 = ps_a.tile([128, 512], F32, tag="psA")
                        a1 = ps_a.tile([128, 512], F32, tag="psA")
                        v0 = ps_pv.tile([128, 512], F32, tag="pv")
                        v1 = ps_pv.tile([128, 512], F32, tag="pv")
                        return [
                            sc0[:, 0:512], sc0[:, 512:1024],
                            sc1[:, 0:512], sc1[:, 512:1024],
                            a0[:], a1[:], v0[:], v1[:],
                        ]

                    # QT(0): paced by the wq/x0 stream
                    slots = alloc_slots8()
                    qt_t[0] = {}
                    for c in range(NCHUNK_Q):
                        for j in range(NPAIR):
                            nc.tensor.matmul(
                                slots[j],
                                wq_sb[:, c * INNER + j * 128 : c * INNER + (j + 1) * 128],
                                xt_t[0][c],
                                start=(c == 0),
                                stop=(c == NCHUNK_Q - 1),
                            )
                    for j in range(NPAIR):
                        qt_t[0][j] = qp.tile([128, 512], F32R, tag="qt", name=f"qt0_{j}")
                        if j % 2 == 0:
                            nc.vector.tensor_copy(qt_t[0][j][:], slots[j])
                        else:
                            nc.scalar.copy(out=qt_t[0][j][:], in_=slots[j])

                    # KT per head pair [128 (2 heads d), 512 m]: wk/ctx-paced
                    slots = alloc_slots8()
                    for c in range(NCHUNK_K):
                        for j in range(NPAIR):
                            nc.tensor.matmul(
                                slots[j],
                                wk_sb[:, c * INNER + j * 128 : c * INNER + (j + 1) * 128],
                                ctx_sb[:, c * M : (c + 1) * M],
                                start=(c == 0),
                                stop=(c == NCHUNK_K - 1),
                            )
                    for j in range(NPAIR):
                        # alternate engines: the serial copy chain is what
                        # frees the V projection's psum slots
                        if j % 2 == 0:
                            nc.scalar.copy(
                                out=kt_sb[:, j * M : (j + 1) * M], in_=slots[j]
                            )
                        else:
                            nc.vector.tensor_copy(
                                kt_sb[:, j * M : (j + 1) * M], slots[j]
                            )

                    # V natural [m, d] with ones col, 8 (mc, hf) units. wv is
                    # resident by now, so per-unit order paces with the KT
                    # copies that progressively free the psum ring slots.
                    slots = alloc_slots8()
                    units = [(mc, hf) for hf in range(2) for mc in range(NMC)]
                    for u, (mc, hf) in enumerate(units):
                        for c in range(NCHUNK_K):
                            nc.tensor.matmul(
                                slots[u],
                                ctx_sb[:, c * M + mc * 128 : c * M + (mc + 1) * 128],
                                wv_sb[:, c * INNER + hf * 512 : c * INNER + (hf + 1) * 512],
                                start=(c == 0),
                                stop=(c == NCHUNK_K - 1),
                            )
                        base = mc * H * VBLK + hf * 8 * VBLK
                        nc.vector.tensor_copy(
                            v_sb[:, base : base + 8 * VBLK].rearrange(
                                "p (h q) -> p h q", q=VBLK
                            )[:, :, 0:D],
                            slots[u].rearrange("p (h q) -> p h q", q=D),
                        )

                    # bias broadcast on the idle Pool engine
                    nc.gpsimd.partition_broadcast(bo_sb[:, :], bo_row[0:1, :])
                    nc.scalar.copy(out=bo_row_r[:], in_=bo_row[:])

                # ---- phase B: software-pipelined across 512-query tiles ----
                with (
                    tc.tile_pool(name="et", bufs=6) as ep,
                    tc.tile_pool(name="osb", bufs=2) as op,
                    tc.tile_pool(name="otb", bufs=2) as obp,
                    tc.tile_pool(name="rows", bufs=4) as rp,
                    tc.tile_pool(name="ysb", bufs=4) as yp,
                ):
                    for h in range(2):
                        nc.sync.dma_start(
                            out=wo_sb[:, h * 4 * C : (h + 1) * 4 * C],
                            in_=wo[h * 512 : (h + 1) * 512, :].rearrange(
                                "(c p) f -> p c f", p=128
                            ),
                        )

                    # Wo groups are split: the j0..j6 partial runs at step g,
                    # the j7 contribution + bias + y store at step g+1. No Wo
                    # matmul ever waits on the freshly-transposed pair 7, and
                    # the y DMA's producers are long done when SP issues it
                    # (DMA waits hold the sequencer in this machine).
                    wo_pend = {}

                    def emit_wo_partial(nt, g):
                        ns, cg = g // 2, g % 2
                        otb = otb_t[nt]
                        yps = ps_a.tile([128, 512], F32, tag="psA")
                        for j in range(NPAIR - 1):
                            nc.tensor.matmul(
                                yps[:],
                                otb[:, (ns * 8 + j) * 128 : (ns * 8 + j + 1) * 128],
                                wo_sb[:, j * C + cg * 512 : j * C + (cg + 1) * 512],
                                start=(j == 0),
                                stop=False,
                            )
                        wo_pend[(nt, g)] = yps

                    y_fifo = []
                    wo_ysb = {}

                    def emit_wo_finish(nt, g):
                        ns, cg = g // 2, g % 2
                        otb = otb_t[nt]
                        yps = wo_pend.pop((nt, g))
                        j = NPAIR - 1
                        nc.tensor.matmul(
                            yps[:],
                            otb[:, (ns * 8 + j) * 128 : (ns * 8 + j + 1) * 128],
                            wo_sb[:, j * C + cg * 512 : j * C + (cg + 1) * 512],
                            start=False,
                            stop=True,
                        )
                        # Both cg halves of a row block share one [128,1024]
                        # ysb and go out as a single store, deferred a step
                        # past the second add: half the HWDGE trips, and the
                        # DMA's sem wait (which holds the SP sequencer) is
                        # already satisfied when it issues.
                        if cg == 0:
                            wo_ysb[(nt, ns)] = yp.tile(
                                [128, 1024], BF16, tag="ysb", name=f"ysb{nt}_{ns}"
                            )
                        ysb = wo_ysb[(nt, ns)]
                        nc.vector.tensor_add(
                            ysb[:, cg * 512 : (cg + 1) * 512],
                            yps[:],
                            bo_sb[:, cg * 512 : (cg + 1) * 512],
                        )
                        if cg == 1:
                            y_fifo.append((nt, ns, wo_ysb.pop((nt, ns))))

                    def y_flush(k):
                        for _ in range(min(k, len(y_fifo))):
                            nt, ns, ysb = y_fifo.pop(0)
                            nc.sync.dma_start(
                                out=y[
                                    nt * 512 + ns * 128 : nt * 512 + (ns + 1) * 128,
                                    :,
                                ],
                                in_=ysb[:],
                            )

                    def emit_attn_scores(nt, j, last=False):
                        qt = qt_t[nt][j]
                        # scoresT both halves, mc-interleaved: halves hit PE
                        # row strips 0-63 / 64-127. For the last pair, emit
                        # odd-half-major and split each exp into two half-tile
                        # activations so the tail PV paces per m-chunk instead
                        # of waiting the full 4us exp train.
                        ets = [[], []]
                        order = (
                            [(mcp, half) for half in (1, 0) for mcp in range(2)]
                            if last
                            else [(mcp, half) for mcp in range(2) for half in range(2)]
                        )
                        for mcp, half in order:
                            p0, p1 = half * 64, half * 64 + 64
                            # two m-chunks share a 2-bank psum tile so one
                            # ACT exp covers both (fixed-cost amortize)
                            scps = ps_sc.tile([128, 1024], F32, tag="sc")
                            for k in range(2):
                                mc = 2 * mcp + k
                                nc.tensor.matmul(
                                    scps[:, k * 512 : (k + 1) * 512],
                                    kt_sb[p0:p1, j * M + mc * 128 : j * M + (mc + 1) * 128],
                                    qt[p0:p1, :],
                                    start=True,
                                    stop=True,
                                )
                            et = ep.tile([128, 1024], BF16, tag="et")
                            if last:
                                for k in range(2):
                                    nc.scalar.activation(
                                        et[:, k * 512 : (k + 1) * 512],
                                        scps[:, k * 512 : (k + 1) * 512],
                                        mybir.ActivationFunctionType.Exp,
                                    )
                            else:
                                nc.scalar.activation(
                                    et[:], scps[:], mybir.ActivationFunctionType.Exp
                                )
                            ets[half].append(et)
                        return ets

                    def emit_attn_pvT(nt, j, ets):
                        # Transposed PV per head: O[n128, 65] = sum_mc
                        # ET_slice^T V_aug. Free dim 65 -> the PE streams
                        # ~half the columns of the natural orientation, and
                        # col 64 lands the softmax denominator per PARTITION
                        # so normalization is a DVE tensor_scalar. O is one
                        # nk-major [128, 4096] tile per nt so a single XBAR
                        # transpose instruction covers the whole n-tile.
                        if j == 0:
                            osb_t[nt] = op.tile(
                                [128, 4096], BF16, tag="osb", name=f"osb{nt}"
                            )
                        single = j == JL or (nt == NT - 1 and j >= JL - 2)
                        if single:
                            # transpose-latency-critical pairs (every pair 7,
                            # plus the last n-tile's pairs 5/6) land in their
                            # own contiguous nk-major buffer so each whole
                            # transpose is ONE instruction instead of four
                            osb7_t[(nt, j)] = op.tile(
                                [128, 512], BF16, tag="osb7",
                                name=f"osbp{nt}_{j}", bufs=3,
                            )
                        osb = osb_t[nt]
                        for half in (0, 1):
                            h = 2 * j + half
                            pv = ps_pv.tile([128, 512], F32, tag="pv")
                            for nk in range(4):
                                for mc in range(NMC):
                                    vb = mc * H * VBLK + h * VBLK
                                    etap = ets[half][mc // 2][
                                        :, (mc % 2) * 512 + nk * 128
                                        : (mc % 2) * 512 + (nk + 1) * 128
                                    ]
                                    nc.tensor.matmul(
                                        pv[:, nk * VBLK : (nk + 1) * VBLK],
                                        etap,
                                        v_sb[:, vb : vb + VBLK],
                                        start=(mc == 0),
                                        stop=(mc == NMC - 1),
                                    )
                            # normalize: recip of the 4 denom cols, then 4
                            # per-partition scalar muls into O_sb (bf16)
                            pvv = pv[:, 0 : 4 * VBLK].rearrange(
                                "p (a q) -> p a q", q=VBLK
                            )
                            rr = rp.tile([128, 4], F32, tag="rows")
                            with nc.allow_low_precision(reason="denoms>0"):
                                nc.vector.reciprocal(
                                    rr[:].rearrange("p (a q) -> p a q", q=1),
                                    pvv[:, :, D : D + 1],
                                )
                            if single:
                                nout = osb7_t[(nt, j)][:].rearrange(
                                    "p (a q) -> p a q", q=128
                                )[:, :, half * 64 : (half + 1) * 64]
                            else:
                                nout = osb[:].rearrange(
                                    "p (a q) -> p a q", q=1024
                                )[:, :, h * 64 : (h + 1) * 64]
                            nc.vector.tensor_mul(
                                nout,
                                pvv[:, :, 0:D],
                                rr[:].rearrange("p (a q) -> p a q", q=1)
                                .broadcast_to((128, 4, D)),
                            )

                    def emit_transpose(nt, j0, j1):
                        # O[n, hd] -> OT[hd, n] on the DMA engines' XBAR. The
                        # otb layout is (nk, j)-major ([hd, (nk*8+j)*128+n])
                        # so a FULL n-tile is ONE instruction (3D out AP:
                        # out[hd, g, n] = in[n, g*128+hd]); pair ranges are
                        # one instr per n-chunk. Wo reads its lhsT slice at
                        # (ns*8+j)*128, which is just a re-indexing.
                        if nt not in otb_t:
                            otb_t[nt] = obp.tile(
                                [128, NPAIR * 512], BF16, tag="otb",
                                name=f"otb{nt}",
                            )
                        otb = otb_t[nt]
                        if j1 - j0 == 1 and (nt, j0) in osb7_t:
                            # one instruction from the pair's contiguous
                            # nk-major buffer; out blocks stride 1024 apart
                            nc.sync.dma_start_transpose(
                                out=otb[:].rearrange("p (a q) -> p a q", q=1024)[
                                    :, :, j0 * 128 : (j0 + 1) * 128
                                ],
                                in_=osb7_t[(nt, j0)][:],
                            )
                            return
                        osb = osb_t[nt]
                        for nk in range(4):
                            nc.sync.dma_start_transpose(
                                out=otb[
                                    :, (nk * 8 + j0) * 128 : (nk * 8 + j1) * 128
                                ].rearrange("p (g n) -> p g n", n=128),
                                in_=osb[:, nk * 1024 + j0 * 128 : nk * 1024 + j1 * 128],
                            )

                    JL = NPAIR - 1
                    for nt in range(NT):
                        if nt + 2 < NT:
                            emit_x_nt(nt + 2)
                        for j in range(NPAIR):
                            if nt == NT - 1 and j == JL:
                                break  # tail handled below
                            ets = emit_attn_scores(nt, j)
                            # the previous group's j7 finisher goes right
                            # after the scores: its DVE add is what recycles
                            # the psA slot the upcoming partial needs, so it
                            # must not queue behind this pair's normalize
                            if nt > 0:
                                if j >= 1:
                                    emit_wo_finish(nt - 1, j - 1)
                                elif nt >= 2:
                                    emit_wo_finish(nt - 2, JL)
                                y_flush(1)
                            emit_attn_pvT(nt, j, ets)
                            # Pairs 0..6 transpose at their own nt's step 7
                            # (all normalized by then), pair 7 at the next
                            # nt's step 0 — so the boundary Wo partial (which
                            # reads j0..6) never waits on an in-flight
                            # transpose, and the j7 finisher has a full step
                            # of slack. nk-ascending instrs mean group g only
                            # needs instr nk=g//2.
                            if j == 0 and nt > 0:
                                emit_transpose(nt - 1, JL, NPAIR)  # pair 7
                            if j == JL and nt < NT - 1:
                                emit_transpose(nt, 0, JL)  # pairs 0..6
                            if nt == NT - 1 and j == JL - 2:
                                emit_transpose(nt, 0, JL - 2)  # pairs 0..4
                            if nt == NT - 1 and j == JL - 1:
                                emit_transpose(nt, JL - 2, JL - 1)  # pair 5
                            if nt + 1 < NT:
                                emit_qt_pair(nt + 1, j)
                            if nt > 0:
                                emit_wo_partial(nt - 1, j)
                    emit_transpose(NT - 1, JL - 1, JL)  # pair 6 of the last nt
                    # dangling Wo(2) work the broken step (3,7) would have run
                    emit_wo_finish(NT - 2, JL - 1)
                    y_flush(2)
                    emit_wo_partial(NT - 2, JL)
                    emit_wo_finish(NT - 2, JL)
                    y_flush(1)

                    # ---- tail: last pair of the last n-tile ----
                    # Fill the exp/normalize latency with j0..j6 Wo partials.
                    # Only 2 psA slots exist, so the remaining partials borrow
                    # the score/PV psum rings (idle from here on); all eight
                    # j7 finishers then fire back-to-back once ot(3,7) lands.
                    ets = emit_attn_scores(NT - 1, JL)
                    wps = []
                    for g in range(2):
                        wt = ps_a.tile([128, 512], F32, tag="psA", name=f"tailA{g}")
                        wps.append(wt[:])

                    def tail_partial(g, j0=0, j1=NPAIR - 1):
                        ns, cg = g // 2, g % 2
                        otb = otb_t[NT - 1]
                        for j in range(j0, j1):
                            nc.tensor.matmul(
                                wps[g],
                                otb[:, (ns * 8 + j) * 128 : (ns * 8 + j + 1) * 128],
                                wo_sb[:, j * C + cg * 512 : j * C + (cg + 1) * 512],
                                start=(j == 0),
                                stop=False,
                            )


                    tail_ysb = {}

                    def tail_finish(g):
                        # j7 contribution, then bias via a PE outer product
                        # into the same accumulation. cg pairs share one
                        # [128,1024] ysb so each row-block needs a single
                        # HWDGE hold at the very end instead of two.
                        ns, cg = g // 2, g % 2
                        otb = otb_t[NT - 1]
                        nc.tensor.matmul(
                            wps[g],
                            otb[:, (ns * 8 + JL) * 128 : (ns * 8 + JL + 1) * 128],
                            wo_sb[:, JL * C + cg * 512 : JL * C + (cg + 1) * 512],
                            start=False,
                            stop=False,
                        )
                        nc.tensor.matmul(
                            wps[g],
                            ones_r[0:1, :],
                            bo_row_r[0:1, cg * 512 : (cg + 1) * 512],
                            start=False,
                            stop=True,
                        )
                        if ns not in tail_ysb:
                            # share the main ysb ring (4 deep) so this
                            # allocation never waits on an earlier y store
                            tail_ysb[ns] = yp.tile(
                                [128, 1024], BF16, tag="ysb", name=f"ysbw{ns}"
                            )
                        ysb = tail_ysb[ns]
                        # spread the PSUM->SBUF hops so neither engine
                        # serializes the closing stores (ACT takes the g5/g7
                        # halves that gate the last two row blocks)
                        if g in (0, 2, 4, 5, 7):
                            nc.scalar.copy(
                                out=ysb[:, cg * 512 : (cg + 1) * 512], in_=wps[g]
                            )
                        else:
                            nc.vector.tensor_copy(
                                ysb[:, cg * 512 : (cg + 1) * 512], wps[g]
                            )
                        if cg == 1:
                            nc.sync.dma_start(
                                out=y[
                                    (NT - 1) * 512 + ns * 128 : (NT - 1) * 512 + (ns + 1) * 128,
                                    :,
                                ],
                                in_=ysb[:],
                            )

                    # Partials run in two rounds: j0..j4 (whose OT chunks are
                    # long since resident) fill the PE while the last pair's
                    # exp/PV/normalize chain runs; j5..j6 follow once the
                    # late transposes land. Keeping the rounds as separate
                    # accumulation bursts means no in-order PE chain ever
                    # head-of-line blocks on a transpose still in flight.
                    tail_partial(0, 0, NPAIR - 3)
                    tail_partial(1, 0, NPAIR - 3)
                    emit_attn_pvT(NT - 1, JL, ets)
                    emit_transpose(NT - 1, JL, NPAIR)
                    # sc/pv ring slots must be claimed after the last pair's
                    # scores and PV tiles so ring reuse ordering cannot
                    # deadlock against the finishers
                    sc0 = ps_sc.tile([128, 1024], F32, tag="sc")
                    sc1 = ps_sc.tile([128, 1024], F32, tag="sc")
                    wps += [sc0[:, 0:512], sc0[:, 512:1024],
                            sc1[:, 0:512], sc1[:, 512:1024]]
                    for g in range(2):
                        wt = ps_pv.tile([128, 512], F32, tag="pv", name=f"tailPV{g}")
                        wps.append(wt[:])
                    for g in range(2, NPAIR):
                        tail_partial(g, 0, NPAIR - 3)
                    # round 2 + finishers, interleaved so each finisher fires
                    # as soon as its ot(3,7) chunk lands
                    tail_partial(0, NPAIR - 3, NPAIR - 1)
                    tail_partial(1, NPAIR - 3, NPAIR - 1)
                    for g in range(2, NPAIR):
                        tail_partial(g, NPAIR - 3, NPAIR - 1)
                        tail_finish(g - 2)
                    tail_finish(NPAIR - 2)
                    tail_finish(NPAIR - 1)

    nc.compile()
    return nc


_NC_CACHE = None


def kernel(x, context, Wq, Wk, Wv, Wo, bo, _trace=False, _trace_kwargs=None):
    global _NC_CACHE
    if _NC_CACHE is None:
        _NC_CACHE = build_nc()
    nc = _NC_CACHE

    bf16 = mybir.dt.np(BF16)
    x = np.asarray(x, np.float32)
    context = np.asarray(context, np.float32)
    wq_s = (np.asarray(Wq, np.float32) * np.float32(D**-0.5)).astype(bf16)
    wk = np.asarray(Wk, np.float32).astype(bf16)
    wv = np.asarray(Wv, np.float32).astype(bf16)
    wo = np.asarray(Wo, np.float32).astype(bf16)
    bo2 = np.asarray(bo, np.float32).reshape(1, C)

    in_maps = []
    for i in range(8):
        b, hf = i // 2, i % 2
        in_maps.append(
            {
                "xT": np.ascontiguousarray(x[b, hf * NPC : (hf + 1) * NPC, :].T).astype(bf16),
                "ctxT": np.ascontiguousarray(context[b].T).astype(bf16),
                "wq": wq_s,
                "wk": wk,
                "wv": wv,
                "wo": wo,
                "bo": bo2,
            }
        )

    kw = {}
    if _trace:
        kw = dict(trace=True, trace_kwargs=_trace_kwargs or {})
    res = run_bass_kernel_spmd(nc, in_maps, list(range(8)), **kw)

    out = np.empty((B, N, C), np.float32)
    for i in range(8):
        b, hf = i // 2, i % 2
        out[b, hf * NPC : (hf + 1) * NPC, :] = np.asarray(
            res.results[i]["y"], dtype=np.float32
        )
    if _trace:
        return out, res
    return out
